# revision 20
# baseline (speedup 1.0000x reference)
"""BiLSTM (B=32, S=512, I=H=1024) Trainium2 kernel over 8 NeuronCores.

v2.  Tensor-parallel over the gate dimension (each core owns a 128-row
H-slice and its four gate blocks [i|f|o|g]); both directions interleaved on
all 8 cores.  Per step each core computes its (32, 512) gate slice as
float32r matmuls accumulated in PSUM (xp injected via an identity matmul),
applies sigmoid/tanh on the scalar engine, updates its c/h slice on the DVE,
PE-transposes the h-slice and exchanges it with the other cores through a
per-direction AllGather.  The two directions use differently-ordered replica
groups so the runtime can give them independent collective streams; the
backward direction's recurrent weights are chunk-permuted on the host to
match its gather order.  x_proj is computed on-device from a host-
pretransposed xT into DRAM scratch (one tensor per direction), with an
explicit semaphore ordering phase-C reads after phase-B writes.
"""

S_FIXED = 512

LAST_EXEC_NS = None

import numpy as np

import concourse.bass as bass
import concourse.bacc as bacc
import concourse.mybir as mybir
import concourse.tile as tile
from concourse.tile_rust import add_dep_helper

# The axon client has no /dev/neuron*, so the driver's NC/routing maps are
# unavailable.  A plausible identity map is fine for client-side validation
# and the simulator; the runtime resolves real routing at NEFF load.
import concourse.libnrt as _libnrt

try:
    _libnrt.get_trn2_nc_mapping()
except Exception:
    _libnrt.get_trn2_nc_mapping = lambda: {(0, i): i for i in range(8)}
try:
    _libnrt.get_device_id_to_routing_id_mapping()
except Exception:
    _fake_rid_map = lambda: {i: i for i in range(16)}
    _libnrt.get_device_id_to_routing_id_mapping = _fake_rid_map
    import concourse.bass_interp as _bi
    import concourse.replica_groups as _rg

    _bi.get_device_id_to_routing_id_mapping = _fake_rid_map
    _rg.get_device_id_to_routing_id_mapping = _fake_rid_map

P = 128
B = 32
I_DIM = 1024
H_DIM = 1024
NCORES = 8
KCH = H_DIM // P          # 8 k-chunks of the hidden dim
GS = 4 * H_DIM // NCORES  # 512 gate rows per core
F32 = mybir.dt.float32
F32R = mybir.dt.float32r
BF16 = mybir.dt.bfloat16
SIG = mybir.ActivationFunctionType.Sigmoid
TANH = mybir.ActivationFunctionType.Tanh

# AllGather replica groups per direction.  Same membership, different order:
# distinct orders can be serviced by distinct collective streams.  The gather
# output block r holds the h-slice of core GROUPS[d][r], so the recurrent
# weights for direction d are chunk-permuted accordingly on the host.
GROUPS = {"f": list(range(NCORES)), "b": list(range(NCORES))}


def host_prep(x, W_ii, W_hi, b_i, W_ii_r, W_hi_r, b_i_r, S, rotate=False):
    """Build the 8 per-core input maps."""
    x = np.asarray(x, np.float32)
    # xT[i, s*B+b] = x[b, s, i]
    xT = np.ascontiguousarray(x.transpose(2, 1, 0).reshape(I_DIM, S * B))

    def slices(W, bvec, core):
        # gate rows for this core, in-slice order [i|f|o|g]
        rows_i = np.arange(core * P, core * P + P)
        rows_f = H_DIM + rows_i
        rows_g = 2 * H_DIM + rows_i
        rows_o = 3 * H_DIM + rows_i
        rows = np.concatenate([rows_i, rows_f, rows_o, rows_g])
        Ws = W[rows, :].astype(np.float32)
        bs = bvec[rows].astype(np.float32)
        # transpose -> (K, GS)
        return np.ascontiguousarray(Ws.T), bs.reshape(1, GS).copy()

    id32 = np.eye(B, dtype=np.float32)
    ones128 = np.ones((1, P), dtype=np.float32)
    in_maps = []
    for c in range(NCORES):
        wiT, bias = slices(np.asarray(W_ii), np.asarray(b_i), c)
        whT, _ = slices(np.asarray(W_hi), np.asarray(b_i), c)
        wiT_r, bias_r = slices(np.asarray(W_ii_r), np.asarray(b_i_r), c)
        whT_r, _ = slices(np.asarray(W_hi_r), np.asarray(b_i_r), c)

        def permute_k(w, d):
            grp = GROUPS[d] if rotate else GROUPS["f"]
            import ml_dtypes
            out = np.ascontiguousarray(
                w.reshape(KCH, P, GS)[grp].reshape(H_DIM, GS))
            return out.astype(ml_dtypes.bfloat16)

        in_maps.append({
            "xT": xT,
            "wiT_f": wiT, "whT_f": permute_k(whT, "f"), "bias_f": bias,
            "wiT_b": wiT_r, "whT_b": permute_k(whT_r, "b"), "bias_b": bias_r,
            "id32": id32, "id32f": id32, "ones128": ones128,
        })
    return in_maps


def host_assemble(results, S):
    """results[c]["out"]: (2, S, B, P) -> full (B, S, 2H)."""
    out = np.empty((B, S, 2 * H_DIM), np.float32)
    for c in range(NCORES):
        o = results[c]["out"]  # (2, S, B, P)
        out[:, :, c * P:(c + 1) * P] = o[0].transpose(1, 0, 2)
        out[:, :, H_DIM + c * P:H_DIM + (c + 1) * P] = o[1].transpose(1, 0, 2)
    return out


def build_kernel(S, comm="collective", rel_wait=False, dump_xp=False,
                 rotate=False):
    """Emit the SPMD kernel; returns nc."""
    nc = bacc.Bacc(None)
    SB = S * B
    MCH = SB // P  # sb-chunks of 128 (= 4 timesteps each)

    xT_e = nc.declare_dram_parameter("xT", [I_DIM, SB], F32R, isOutput=False)
    w_e = {}
    for d in ("f", "b"):
        w_e["wiT_" + d] = nc.declare_dram_parameter("wiT_" + d, [I_DIM, GS], F32R, isOutput=False)
        w_e["whT_" + d] = nc.declare_dram_parameter("whT_" + d, [H_DIM, GS], BF16, isOutput=False)
        w_e["bias_" + d] = nc.declare_dram_parameter("bias_" + d, [1, GS], F32R, isOutput=False)
    id32_e = nc.declare_dram_parameter("id32", [B, B], F32R, isOutput=False)
    ones_e = nc.declare_dram_parameter("ones128", [1, P], F32R, isOutput=False)
    id32f_e = nc.declare_dram_parameter("id32f", [B, B], F32, isOutput=False)
    out_e = nc.declare_dram_parameter("out", [2, S, B, P], F32, isOutput=True)
    xp_dump_e = None
    if dump_xp:
        xp_dump_e = nc.declare_dram_parameter(
            "xp_dump", [2, S, B, GS], F32, isOutput=True)

    # One scratch tensor per direction (keeps intra-tensor offsets < 32 MiB).
    xp_dirs = [
        nc.dram_tensor("xp_scratch_f", [S, B, GS], F32R),
        nc.dram_tensor("xp_scratch_b", [S, B, GS], F32R),
    ]
    # Phase-B xp write instructions, keyed (di, m); phase-C reads take an
    # explicit sync dep on the matching write (insurance for the DRAM RAW
    # hazard across the phase boundary).
    xp_writes = {}

    with tile.TileContext(nc) as tc:
        with (
            tc.tile_pool(name="const", bufs=1) as constp,
            tc.tile_pool(name="xsb", bufs=3) as xsbp,
            tc.tile_pool(name="psumB", bufs=2, space="PSUM") as psumB,
            tc.tile_pool(name="psumC", bufs=1, space="PSUM") as psumC,
            tc.tile_pool(name="psumT", bufs=1, space="PSUM") as psumT,
            tc.tile_pool(name="state", bufs=1) as statep,
            tc.tile_pool(name="step", bufs=3) as stepp,
            tc.tile_pool(name="hcomm", bufs=2) as hcommp,
            tc.tile_pool(name="dram", bufs=2, space="DRAM") as dramp,
        ):
            # ---- constants / weights in SBUF ----
            id32 = constp.tile([B, B], F32R, tag="id32", name="id32")
            id32f = constp.tile([B, B], F32, tag="id32f", name="id32f")
            nc.sync.dma_start(id32f[:], id32f_e[:])
            nc.sync.dma_start(id32[:], id32_e[:])
            ones128 = constp.tile([1, P], F32R, tag="ones", name="ones")
            nc.sync.dma_start(ones128[:], ones_e[:])
            wiT = {}
            whT = {}
            biasT = {}
            for d in ("f", "b"):
                wiT[d] = constp.tile([P, KCH, GS], F32R, tag="wiT" + d, name="wiT" + d)
                nc.sync.dma_start(
                    wiT[d][:],
                    w_e["wiT_" + d][:].rearrange("(k p) g -> p k g", p=P),
                )
                whT[d] = constp.tile([P, KCH, GS], BF16, tag="whT" + d, name="whT" + d)
                nc.sync.dma_start(
                    whT[d][:],
                    w_e["whT_" + d][:].rearrange("(k p) g -> p k g", p=P),
                )
                biasT[d] = constp.tile([1, GS], F32R, tag="bias" + d, name="bias" + d)
                nc.sync.dma_start(biasT[d][:], w_e["bias_" + d][:])

            # ---- phase B: x_proj into DRAM xp ----
            def xproj_chunk(m):
                xsb = xsbp.tile([P, KCH, P], F32R, tag="xsb", name="xsb")
                nc.sync.dma_start(
                    xsb[:],
                    xT_e[:, m * P:(m + 1) * P].rearrange("(k p) c -> p k c", p=P),
                )
                for d in ("f", "b"):
                    ps = psumB.tile([P, GS], F32, tag="psB", name="psB")
                    nc.tensor.matmul(ps[:], ones128[:], biasT[d][:],
                                     start=True, stop=False)
                    for k in range(KCH):
                        nc.tensor.matmul(ps[:], xsb[:, k, :], wiT[d][:, k, :],
                                         start=False, stop=(k == KCH - 1))
                    xpt = xsbp.tile([P, GS], F32R, tag="xpt", name="xpt")
                    nc.vector.tensor_copy(xpt[:], ps[:])
                    di = 0 if d == "f" else 1
                    s0 = m * 4
                    xp_writes[(di, m)] = nc.sync.dma_start(
                        xp_dirs[di][s0:s0 + 4].rearrange("s b g -> (s b) g"),
                        xpt[:],
                    )
                    if xp_dump_e is not None:
                        nc.sync.dma_start(
                            xp_dump_e[di, s0:s0 + 4].rearrange("s b g -> (s b) g"),
                            xpt[:].bitcast(F32),
                        )

            for m in range(MCH):
                xproj_chunk(m)

            # ---- phase C: recurrence ----
            c_state = {d: statep.tile([B, P], F32, tag="c_" + d, name="c_" + d)
                       for d in ("f", "b")}
            for d in ("f", "b"):
                nc.vector.memset(c_state[d][:], 0.0)

            hT_prev = {}

            def step(d, t):
                di = 0 if d == "f" else 1
                spos = t if d == "f" else S - 1 - t
                xpt = stepp.tile([B, GS], F32R, tag="xp_t" + d, name="xp_t" + d)
                ld = nc.sync.dma_start(xpt[:], xp_dirs[di][spos])
                add_dep_helper(ld.ins, xp_writes[(di, spos // 4)].ins,
                               sync=True,
                               reason="xp RAW: read after phase-B write")
                ps = psumC.tile([B, GS], F32, tag="psC" + d, name="psC" + d)
                # two 256-col accumulation halves ([i|f] then [o|g]) so the
                # first half's activation + c-update pipeline under the
                # second half's matmuls
                H2 = 2 * P
                for lo in (0, H2):
                    sl = slice(lo, lo + H2)
                    nc.tensor.matmul(ps[:, sl], id32[:], xpt[:, sl],
                                     start=True, stop=(t == 0))
                    if t > 0:
                        hT = hT_prev[d]
                        for k in range(KCH):
                            nc.tensor.matmul(
                                ps[:, sl], hT[:, k, :], whT[d][:, k, sl],
                                start=False, stop=(k == KCH - 1))
                acts = stepp.tile([B, GS], F32, tag="acts" + d, name="acts" + d)
                nc.scalar.activation(acts[:, 0:H2], ps[:, 0:H2], SIG)
                nc.scalar.activation(acts[:, 3 * P:], ps[:, 3 * P:], TANH)
                nc.scalar.activation(acts[:, H2:3 * P], ps[:, H2:3 * P], SIG)
                i_ap = acts[:, 0 * P:1 * P]
                f_ap = acts[:, 1 * P:2 * P]
                o_ap = acts[:, 2 * P:3 * P]
                g_ap = acts[:, 3 * P:4 * P]
                u = stepp.tile([B, P], F32, tag="u" + d, name="u" + d)
                nc.vector.tensor_mul(u[:], i_ap, g_ap)
                v = stepp.tile([B, P], F32, tag="v" + d, name="v" + d)
                nc.vector.tensor_mul(v[:], f_ap, c_state[d][:])
                nc.vector.tensor_add(c_state[d][:], u[:], v[:])
                tc_t = stepp.tile([B, P], F32, tag="tc" + d, name="tc" + d)
                nc.scalar.activation(tc_t[:], c_state[d][:], TANH)
                h = stepp.tile([B, P], F32, tag="h" + d, name="h" + d)
                nc.vector.tensor_mul(h[:], o_ap, tc_t[:])
                nc.sync.dma_start(out_e[di, spos], h[:])
                if t == S - 1:
                    return None
                # transpose h -> (P, B), ship PSUM -> DRAM -> AllGather
                tp = psumT.tile([P, B], F32, tag="tp" + d, name="tp" + d)
                nc.tensor.transpose(tp[:], h[:], id32f[:])
                hself = stepp.tile([P, B], BF16, tag="hself" + d,
                                   name="hself" + d)
                nc.vector.tensor_copy(hself[:], tp[:])
                cc_in = dramp.tile([P, B], BF16, tag="ccin" + d, name="ccin" + d)
                nc.sync.dma_start(cc_in[:], hself[:])
                cc_out = dramp.tile([NCORES * P, B], BF16, tag="ccout" + d,
                                    name="ccout" + d)
                grp = GROUPS[d] if rotate else GROUPS["f"]
                nc.gpsimd.collective_compute(
                    "AllGather",
                    mybir.AluOpType.bypass,
                    ins=[cc_in[:].opt()],
                    outs=[cc_out[:].opt()],
                    replica_groups=[grp],
                )
                hT_new = hcommp.tile([P, KCH, B], BF16, tag="hrecv" + d,
                                     name="hrecv" + d)
                nc.sync.dma_start(
                    hT_new[:],
                    cc_out[:].rearrange("(k p) b -> p k b", p=P),
                )
                return hT_new

            for t in range(S):
                for d in ("f", "b"):
                    hT_prev[d] = step(d, t)

    return nc


def fix_drain_waits(nc):
    """This walrus build allows only 1 sync-wait per instruction (2 on
    EventSemaphore).  Move excess waits onto EventSemaphore insts placed
    immediately before the instruction on the same engine."""
    ctr = 0
    for fn in nc.m.functions:
        for bb in fn.blocks:
            insts = list(bb.instructions)
            new = []
            changed = False
            for ins in insts:
                si = ins.sync_info
                if (
                    not isinstance(ins, mybir.InstEventSemaphore)
                    and si is not None
                    and len(si.on_wait) > 1
                ):
                    waits = list(si.on_wait)
                    keep, extra = waits[:1], waits[1:]
                    for i in range(0, len(extra), 2):
                        w = mybir.InstEventSemaphore(
                            name=f"I-dwfix-{ctr}",
                            engine=ins.engine,
                            ins=[],
                            outs=[],
                            sync_info=mybir.SyncInfo(
                                on_wait=extra[i : i + 2], on_update=[]
                            ),
                        )
                        ctr += 1
                        new.append(w)
                    ins.sync_info = mybir.SyncInfo(
                        on_wait=keep, on_update=list(si.on_update)
                    )
                    changed = True
                new.append(ins)
            if changed:
                try:
                    bb.instructions = new
                except Exception:
                    bb.instructions.clear()
                    bb.instructions.extend(new)


def kernel(x, W_ii, W_hi, b_i, W_ii_reverse, W_hi_reverse, b_i_reverse):
    """Full inputs in, full (B, S, 2H) output out."""
    import os

    global LAST_EXEC_NS
    import concourse.bass_utils as bu

    bu.upload_artifacts = lambda tmpdir: "local://" + tmpdir
    from concourse.bass_utils import run_bass_kernel_spmd

    S = S_FIXED
    trace = os.environ.get("TRNLSTM_TRACE", "0") == "1"
    rotate = os.environ.get("TRNLSTM_ROTATE", "0") == "1"

    nc = build_kernel(S, rotate=rotate)
    nc.compile()
    fix_drain_waits(nc)
    in_maps = host_prep(x, W_ii, W_hi, b_i,
                        W_ii_reverse, W_hi_reverse, b_i_reverse, S,
                        rotate=rotate)
    res = run_bass_kernel_spmd(nc, in_maps, list(range(NCORES)), trace=trace)
    LAST_EXEC_NS = res.exec_time_ns
    return host_assemble(res.results, S)


# revision 22
# speedup vs baseline: 1.1709x; 1.1709x over previous
"""BiLSTM (B=32, S=512, I=H=1024) Trainium2 kernel over 8 NeuronCores.

v2.  Tensor-parallel over the gate dimension (each core owns a 128-row
H-slice and its four gate blocks [i|f|o|g]); both directions interleaved on
all 8 cores.  Per step each core computes its (32, 512) gate slice as
float32r matmuls accumulated in PSUM (xp injected via an identity matmul),
applies sigmoid/tanh on the scalar engine, updates its c/h slice on the DVE,
PE-transposes the h-slice and exchanges it with the other cores through a
per-direction AllGather.  The two directions use differently-ordered replica
groups so the runtime can give them independent collective streams; the
backward direction's recurrent weights are chunk-permuted on the host to
match its gather order.  x_proj is computed on-device from a host-
pretransposed xT into DRAM scratch (one tensor per direction), with an
explicit semaphore ordering phase-C reads after phase-B writes.
"""

S_FIXED = 512

LAST_EXEC_NS = None

import numpy as np

import concourse.bass as bass
import concourse.bacc as bacc
import concourse.mybir as mybir
import concourse.tile as tile
from concourse.tile_rust import add_dep_helper

# The axon client has no /dev/neuron*, so the driver's NC/routing maps are
# unavailable.  A plausible identity map is fine for client-side validation
# and the simulator; the runtime resolves real routing at NEFF load.
import concourse.libnrt as _libnrt

try:
    _libnrt.get_trn2_nc_mapping()
except Exception:
    _libnrt.get_trn2_nc_mapping = lambda: {(0, i): i for i in range(8)}
try:
    _libnrt.get_device_id_to_routing_id_mapping()
except Exception:
    _fake_rid_map = lambda: {i: i for i in range(16)}
    _libnrt.get_device_id_to_routing_id_mapping = _fake_rid_map
    import concourse.bass_interp as _bi
    import concourse.replica_groups as _rg

    _bi.get_device_id_to_routing_id_mapping = _fake_rid_map
    _rg.get_device_id_to_routing_id_mapping = _fake_rid_map

P = 128
B = 32
I_DIM = 1024
H_DIM = 1024
NCORES = 8
KCH = H_DIM // P          # 8 k-chunks of the hidden dim
GS = 4 * H_DIM // NCORES  # 512 gate rows per core
F32 = mybir.dt.float32
F32R = mybir.dt.float32r
BF16 = mybir.dt.bfloat16
SIG = mybir.ActivationFunctionType.Sigmoid
TANH = mybir.ActivationFunctionType.Tanh

# AllGather replica groups per direction.  Same membership, different order:
# distinct orders can be serviced by distinct collective streams.  The gather
# output block r holds the h-slice of core GROUPS[d][r], so the recurrent
# weights for direction d are chunk-permuted accordingly on the host.
GROUPS = {"f": list(range(NCORES)), "b": list(range(NCORES))}


def host_prep(x, W_ii, W_hi, b_i, W_ii_r, W_hi_r, b_i_r, S, rotate=False):
    """Build the 8 per-core input maps."""
    x = np.asarray(x, np.float32)
    # xT[i, s*B+b] = x[b, s, i]
    xT = np.ascontiguousarray(x.transpose(2, 1, 0).reshape(I_DIM, S * B))

    def slices(W, bvec, core):
        # gate rows for this core, in-slice order [i|f|o|g]
        rows_i = np.arange(core * P, core * P + P)
        rows_f = H_DIM + rows_i
        rows_g = 2 * H_DIM + rows_i
        rows_o = 3 * H_DIM + rows_i
        rows = np.concatenate([rows_i, rows_f, rows_o, rows_g])
        Ws = W[rows, :].astype(np.float32)
        bs = bvec[rows].astype(np.float32)
        # transpose -> (K, GS)
        return np.ascontiguousarray(Ws.T), bs.reshape(1, GS).copy()

    id32 = np.eye(B, dtype=np.float32)
    ones128 = np.ones((1, P), dtype=np.float32)
    in_maps = []
    for c in range(NCORES):
        wiT, bias = slices(np.asarray(W_ii), np.asarray(b_i), c)
        whT, _ = slices(np.asarray(W_hi), np.asarray(b_i), c)
        wiT_r, bias_r = slices(np.asarray(W_ii_r), np.asarray(b_i_r), c)
        whT_r, _ = slices(np.asarray(W_hi_r), np.asarray(b_i_r), c)

        def permute_k(w, d):
            grp = GROUPS[d] if rotate else GROUPS["f"]
            import ml_dtypes
            out = np.ascontiguousarray(
                w.reshape(KCH, P, GS)[grp].reshape(H_DIM, GS))
            return out.astype(ml_dtypes.bfloat16)

        in_maps.append({
            "xT": xT,
            "wiT_f": wiT, "whT_f": permute_k(whT, "f"), "bias_f": bias,
            "wiT_b": wiT_r, "whT_b": permute_k(whT_r, "b"), "bias_b": bias_r,
            "id32": id32, "id32f": id32, "ones128": ones128,
        })
    return in_maps


def host_assemble(results, S):
    """results[c]["out"]: (2, S, B, P) -> full (B, S, 2H)."""
    out = np.empty((B, S, 2 * H_DIM), np.float32)
    for c in range(NCORES):
        o = results[c]["out"]  # (2, S, B, P)
        out[:, :, c * P:(c + 1) * P] = o[0].transpose(1, 0, 2)
        out[:, :, H_DIM + c * P:H_DIM + (c + 1) * P] = o[1].transpose(1, 0, 2)
    return out


def build_kernel(S, comm="collective", rel_wait=False, dump_xp=False,
                 rotate=False):
    """Emit the SPMD kernel; returns nc."""
    nc = bacc.Bacc(None)
    SB = S * B
    MCH = SB // P  # sb-chunks of 128 (= 4 timesteps each)

    xT_e = nc.declare_dram_parameter("xT", [I_DIM, SB], F32R, isOutput=False)
    w_e = {}
    for d in ("f", "b"):
        w_e["wiT_" + d] = nc.declare_dram_parameter("wiT_" + d, [I_DIM, GS], F32R, isOutput=False)
        w_e["whT_" + d] = nc.declare_dram_parameter("whT_" + d, [H_DIM, GS], BF16, isOutput=False)
        w_e["bias_" + d] = nc.declare_dram_parameter("bias_" + d, [1, GS], F32R, isOutput=False)
    id32_e = nc.declare_dram_parameter("id32", [B, B], F32R, isOutput=False)
    ones_e = nc.declare_dram_parameter("ones128", [1, P], F32R, isOutput=False)
    id32f_e = nc.declare_dram_parameter("id32f", [B, B], F32, isOutput=False)
    out_e = nc.declare_dram_parameter("out", [2, S, B, P], F32, isOutput=True)
    xp_dump_e = None
    if dump_xp:
        xp_dump_e = nc.declare_dram_parameter(
            "xp_dump", [2, S, B, GS], F32, isOutput=True)

    # One scratch tensor per direction (keeps intra-tensor offsets < 32 MiB).
    xp_dirs = [
        nc.dram_tensor("xp_scratch_f", [S, B, GS], F32R),
        nc.dram_tensor("xp_scratch_b", [S, B, GS], F32R),
    ]
    # Phase-B xp write instructions, keyed (di, m); phase-C reads take an
    # explicit sync dep on the matching write (insurance for the DRAM RAW
    # hazard across the phase boundary).
    xp_writes = {}

    with tile.TileContext(nc) as tc:
        with (
            tc.tile_pool(name="const", bufs=1) as constp,
            tc.tile_pool(name="xsb", bufs=3) as xsbp,
            tc.tile_pool(name="psumB", bufs=2, space="PSUM") as psumB,
            tc.tile_pool(name="psumC", bufs=1, space="PSUM") as psumC,
            tc.tile_pool(name="psumT", bufs=1, space="PSUM") as psumT,
            tc.tile_pool(name="state", bufs=1) as statep,
            tc.tile_pool(name="step", bufs=4) as stepp,
            tc.tile_pool(name="hcomm", bufs=3) as hcommp,
            tc.tile_pool(name="dram", bufs=3, space="DRAM") as dramp,
        ):
            # ---- constants / weights in SBUF ----
            id32 = constp.tile([B, B], F32R, tag="id32", name="id32")
            id32f = constp.tile([B, B], F32, tag="id32f", name="id32f")
            nc.sync.dma_start(id32f[:], id32f_e[:])
            nc.sync.dma_start(id32[:], id32_e[:])
            ones128 = constp.tile([1, P], F32R, tag="ones", name="ones")
            nc.sync.dma_start(ones128[:], ones_e[:])
            wiT = {}
            whT = {}
            biasT = {}
            for d in ("f", "b"):
                wiT[d] = constp.tile([P, KCH, GS], F32R, tag="wiT" + d, name="wiT" + d)
                nc.sync.dma_start(
                    wiT[d][:],
                    w_e["wiT_" + d][:].rearrange("(k p) g -> p k g", p=P),
                )
                whT[d] = constp.tile([P, KCH, GS], BF16, tag="whT" + d, name="whT" + d)
                nc.sync.dma_start(
                    whT[d][:],
                    w_e["whT_" + d][:].rearrange("(k p) g -> p k g", p=P),
                )
                biasT[d] = constp.tile([1, GS], F32R, tag="bias" + d, name="bias" + d)
                nc.sync.dma_start(biasT[d][:], w_e["bias_" + d][:])

            # ---- phase B: x_proj into DRAM xp ----
            def xproj_chunk(m):
                xsb = xsbp.tile([P, KCH, P], F32R, tag="xsb", name="xsb")
                nc.sync.dma_start(
                    xsb[:],
                    xT_e[:, m * P:(m + 1) * P].rearrange("(k p) c -> p k c", p=P),
                )
                for d in ("f", "b"):
                    ps = psumB.tile([P, GS], F32, tag="psB", name="psB")
                    nc.tensor.matmul(ps[:], ones128[:], biasT[d][:],
                                     start=True, stop=False)
                    for k in range(KCH):
                        nc.tensor.matmul(ps[:], xsb[:, k, :], wiT[d][:, k, :],
                                         start=False, stop=(k == KCH - 1))
                    xpt = xsbp.tile([P, GS], F32R, tag="xpt", name="xpt")
                    nc.vector.tensor_copy(xpt[:], ps[:])
                    di = 0 if d == "f" else 1
                    s0 = m * 4
                    xp_writes[(di, m)] = nc.sync.dma_start(
                        xp_dirs[di][s0:s0 + 4].rearrange("s b g -> (s b) g"),
                        xpt[:],
                    )
                    if xp_dump_e is not None:
                        nc.sync.dma_start(
                            xp_dump_e[di, s0:s0 + 4].rearrange("s b g -> (s b) g"),
                            xpt[:].bitcast(F32),
                        )

            for m in range(MCH):
                xproj_chunk(m)

            # ---- phase C: recurrence ----
            c_state = {d: statep.tile([B, P], F32, tag="c_" + d, name="c_" + d)
                       for d in ("f", "b")}
            for d in ("f", "b"):
                nc.vector.memset(c_state[d][:], 0.0)

            hT_prev = {}

            def step(d, t):
                di = 0 if d == "f" else 1
                spos = t if d == "f" else S - 1 - t
                xpt = stepp.tile([B, GS], F32R, tag="xp_t" + d, name="xp_t" + d)
                ld = nc.sync.dma_start(xpt[:], xp_dirs[di][spos])
                add_dep_helper(ld.ins, xp_writes[(di, spos // 4)].ins,
                               sync=True,
                               reason="xp RAW: read after phase-B write")
                ps = psumC.tile([B, GS], F32, tag="psC" + d, name="psC" + d)
                nc.tensor.matmul(ps[:], id32[:], xpt[:],
                                 start=True, stop=(t == 0))
                if t > 0:
                    hT = hT_prev[d]
                    for k in range(KCH):
                        nc.tensor.matmul(ps[:], hT[:, k, :], whT[d][:, k, :],
                                         start=False, stop=(k == KCH - 1))
                acts = stepp.tile([B, GS], F32, tag="acts" + d, name="acts" + d)
                nc.scalar.activation(acts[:, 0:3 * P], ps[:, 0:3 * P], SIG)
                nc.scalar.activation(acts[:, 3 * P:], ps[:, 3 * P:], TANH)
                i_ap = acts[:, 0 * P:1 * P]
                f_ap = acts[:, 1 * P:2 * P]
                o_ap = acts[:, 2 * P:3 * P]
                g_ap = acts[:, 3 * P:4 * P]
                u = stepp.tile([B, P], F32, tag="u" + d, name="u" + d)
                nc.vector.tensor_mul(u[:], i_ap, g_ap)
                v = stepp.tile([B, P], F32, tag="v" + d, name="v" + d)
                nc.vector.tensor_mul(v[:], f_ap, c_state[d][:])
                nc.vector.tensor_add(c_state[d][:], u[:], v[:])
                tc_t = stepp.tile([B, P], F32, tag="tc" + d, name="tc" + d)
                nc.scalar.activation(tc_t[:], c_state[d][:], TANH)
                h = stepp.tile([B, P], F32, tag="h" + d, name="h" + d)
                nc.vector.tensor_mul(h[:], o_ap, tc_t[:])
                nc.sync.dma_start(out_e[di, spos], h[:])
                if t == S - 1:
                    return None
                # transpose h -> (P, B), ship PSUM -> DRAM -> AllGather
                tp = psumT.tile([P, B], F32, tag="tp" + d, name="tp" + d)
                nc.tensor.transpose(tp[:], h[:], id32f[:])
                hself = stepp.tile([P, B], BF16, tag="hself" + d,
                                   name="hself" + d)
                nc.vector.tensor_copy(hself[:], tp[:])
                cc_in = dramp.tile([P, B], BF16, tag="ccin" + d, name="ccin" + d)
                nc.sync.dma_start(cc_in[:], hself[:])
                cc_out = dramp.tile([NCORES * P, B], BF16, tag="ccout" + d,
                                    name="ccout" + d)
                grp = GROUPS[d] if rotate else GROUPS["f"]
                nc.gpsimd.collective_compute(
                    "AllGather",
                    mybir.AluOpType.bypass,
                    ins=[cc_in[:].opt()],
                    outs=[cc_out[:].opt()],
                    replica_groups=[grp],
                )
                hT_new = hcommp.tile([P, KCH, B], BF16, tag="hrecv" + d,
                                     name="hrecv" + d)
                nc.sync.dma_start(
                    hT_new[:],
                    cc_out[:].rearrange("(k p) b -> p k b", p=P),
                )
                return hT_new

            for t in range(S):
                for d in ("f", "b"):
                    hT_prev[d] = step(d, t)

    return nc


def fix_drain_waits(nc):
    """This walrus build allows only 1 sync-wait per instruction (2 on
    EventSemaphore).  Move excess waits onto EventSemaphore insts placed
    immediately before the instruction on the same engine."""
    ctr = 0
    for fn in nc.m.functions:
        for bb in fn.blocks:
            insts = list(bb.instructions)
            new = []
            changed = False
            for ins in insts:
                si = ins.sync_info
                if (
                    not isinstance(ins, mybir.InstEventSemaphore)
                    and si is not None
                    and len(si.on_wait) > 1
                ):
                    waits = list(si.on_wait)
                    keep, extra = waits[:1], waits[1:]
                    for i in range(0, len(extra), 2):
                        w = mybir.InstEventSemaphore(
                            name=f"I-dwfix-{ctr}",
                            engine=ins.engine,
                            ins=[],
                            outs=[],
                            sync_info=mybir.SyncInfo(
                                on_wait=extra[i : i + 2], on_update=[]
                            ),
                        )
                        ctr += 1
                        new.append(w)
                    ins.sync_info = mybir.SyncInfo(
                        on_wait=keep, on_update=list(si.on_update)
                    )
                    changed = True
                new.append(ins)
            if changed:
                try:
                    bb.instructions = new
                except Exception:
                    bb.instructions.clear()
                    bb.instructions.extend(new)


def kernel(x, W_ii, W_hi, b_i, W_ii_reverse, W_hi_reverse, b_i_reverse):
    """Full inputs in, full (B, S, 2H) output out."""
    import os

    global LAST_EXEC_NS
    import concourse.bass_utils as bu

    bu.upload_artifacts = lambda tmpdir: "local://" + tmpdir
    from concourse.bass_utils import run_bass_kernel_spmd

    S = S_FIXED
    trace = os.environ.get("TRNLSTM_TRACE", "0") == "1"
    rotate = os.environ.get("TRNLSTM_ROTATE", "0") == "1"

    nc = build_kernel(S, rotate=rotate)
    nc.compile()
    fix_drain_waits(nc)
    in_maps = host_prep(x, W_ii, W_hi, b_i,
                        W_ii_reverse, W_hi_reverse, b_i_reverse, S,
                        rotate=rotate)
    res = run_bass_kernel_spmd(nc, in_maps, list(range(NCORES)), trace=trace)
    LAST_EXEC_NS = res.exec_time_ns
    return host_assemble(res.results, S)
